# revision 10
# baseline (speedup 1.0000x reference)
"""Trainium2 Bass kernel for nn_DecoderLSTMAttention.

Math (exploiting that the reference softmax is over a singleton axis, so
attention weights are identically 1 and context == features broadcast):

    x        = concat([features[:, None, :], embed[captions[:, :-1]]], 1)   # (B,S,E)
    xg       = x @ W_ih.T + (b_ih + b_hh)                                   # (B,S,4H)
    h_t, c_t = lstm_step(xg_t, h_{t-1}, c_{t-1}; W_hh)                      # gates i,f,g,o
    out      = (lstm_out + features[:, None, :]) @ out_W.T + out_b          # (B,S,V)

Sharding: pure data-parallel over batch B=64 across 8 cores (8 batches per
core, no collectives).  Embedding gather + weight transposes/casts happen on
the host; everything else on device.

Device layout (per core, Bc=8, H=1024, G=4H=4096, T=S*Bc tokens, token
index tau = t*Bc + b time-major):
  - xgT    [128, 32, S, 8]  gate-major ("transposed") input gates, bf16
  - gatesT [128, 16, 8] PSUM per (step, half): W_hh^T tiles stationary,
    h_{t-1} moving (N=8).  The recurrence is LDWEIGHTS-bound (every step
    streams all of W_hh into the PE), so W_hh and the recurrent h operand
    are fp8e4m3: FWL loads 4 fp8/cycle vs 2 bf16/cycle.  Final-output
    error is unchanged (2.9e-3) because |h| << |features| and the output
    GEMM stays bf16.
  - gate columns permuted so that half hh occupies m-tiles [16hh,16hh+16)
    as [i,i,i,i, f,f,f,f, o,o,o,o, g,g,g,g] covering hidden cols
    [512hh, 512hh+512) -> elementwise runs hidden-on-partition
  - big GEMM: stationary combT (lstm_out+features, bf16), moving out_W^T
    2048-wide panels (512KB DMAs) streamed from DRAM after the recurrence
    pools free their SBUF, fp32 PSUM -> bf16 SBUF -> DRAM out [T, V]
    (host casts back to fp32)
"""

import numpy as np
import ml_dtypes

import concourse.bass as bass
import concourse.tile as tile
from concourse import bacc, mybir
from concourse.bass_utils import run_bass_kernel_spmd

BF16 = mybir.dt.bfloat16
F32 = mybir.dt.float32
F8 = mybir.dt.float8e4
AF = mybir.ActivationFunctionType

P = 128
BC = 8          # batches per core
H = 1024        # hidden = embed
G = 4 * H       # gates
NK = H // P     # 8 k-chunks
NM = G // P     # 32 gate m-tiles

# permuted gate-tile order: per half-block the gates are laid out [i,f,o,g]
# (so one sigmoid covers [0:12) and tanh covers [12:16)); orig W rows are
# [i,f,g,o] so gate order (0,1,3,2).
PERM_TILES = [gate * 8 + half * 4 + j
              for half in range(2) for gate in (0, 1, 3, 2) for j in range(4)]
PERM = np.concatenate([np.arange(P) + t * P for t in PERM_TILES])


def nv_tiles(V):
    """[(offset, size), ...] vocab tiles of <=512."""
    out = []
    off = 0
    while off < V:
        sz = min(512, V - off)
        out.append((off, sz))
        off += sz
    return out


def emit_body(tc, io, S, V):
    """Emit the per-core program. io maps logical names -> DRAM APs."""
    nc = tc.nc
    T = S * BC
    NMT = T // P          # token m-tiles for the big GEMM
    assert T % P == 0 and T % 2 == 0

    xt_d, wih_d, whh_d, outw_d = io["xt"], io["wih"], io["whh"], io["outw"]
    feat_d, bias_d, out_d = io["feat"], io["biasv"], io["out"]

    import contextlib
    ctx = contextlib.ExitStack()
    with ctx:
        state = ctx.enter_context(tc.tile_pool(name="state", bufs=1))

        # ---- persistent tensors (live through GEMM B) ----
        feat_sb = state.tile([P, NK, BC], F32, tag="feat_sb")
        nc.sync.dma_start(feat_sb[:], feat_d[:])
        bias_sb = state.tile([P, NM], F32, tag="bias_sb")
        nc.sync.dma_start(bias_sb[:], bias_d[:])
        lstm_sb = state.tile([P, NK, S, BC], BF16, tag="lstm_sb")
        comb_sb = state.tile([P, NK, S, BC], BF16, tag="comb_sb")
        c_sb = state.tile([P, NK, BC], F32, tag="c_sb")
        nc.any.memset(c_sb[:], 0.0)
        ident_sb = state.tile([P, P], BF16, tag="ident_sb")
        nc.sync.dma_start(ident_sb[:], io["ident"][:])

        # outw panel pool opens before the recurrence pools: the first two
        # GEMM B blocks prefetch during phases 1+2 (fp8 W_hh leaves the SBUF
        # headroom), so GEMM B starts with zero DMA-wait at the transition.
        outw_pool = ctx.enter_context(tc.tile_pool(name="outw", bufs=16))
        tiles = nv_tiles(V)
        blocks = [tiles[i:i + 4] for i in range(0, len(tiles), 4)]
        pre_panels = {}
        for bi in range(2):
            blk = blocks[bi]
            width = sum(sz for _, sz in blk)
            off0 = blk[0][0]
            row = []
            for kk in range(NK):
                pan = outw_pool.tile([P, 4 * 512], BF16, tag="outw_p",
                                     name="outw_pre")
                nc.sync.dma_start(
                    pan[:, :width],
                    outw_d[kk * P:(kk + 1) * P, off0:off0 + width])
                row.append(pan)
            pre_panels[bi] = row

        # ---- phase 1+2 pools: freed before GEMM B so outw panels get SBUF ----
        with tc.tile_pool(name="rec", bufs=1) as rec_pool, \
             tc.tile_pool(name="wih", bufs=16) as wih_pool, \
             tc.tile_pool(name="gates", bufs=6) as gates_pool, \
             tc.tile_pool(name="tmps", bufs=6) as tmp_pool, \
             tc.tile_pool(name="psA", bufs=2, space="PSUM") as psA_pool, \
             tc.tile_pool(name="gps", bufs=6, space="PSUM") as gps_pool:

            whh_sb = rec_pool.tile([P, NK, G], F8, tag="whh_sb")
            nc.sync.dma_start(whh_sb[:], whh_d.rearrange("(k p) g -> p k g", p=P))
            xt_sb = rec_pool.tile([P, NK, T], BF16, tag="xt_sb")
            nc.sync.dma_start(xt_sb[:], xt_d.rearrange("(k p) t -> p k t", p=P))
            xg_sb = rec_pool.tile([P, NM, S, BC], BF16, tag="xg_sb")
            h8_sb = rec_pool.tile([P, NK, S, BC], F8, tag="h8_sb")

            # ---- GEMM A: xgT[g, tau] = sum_e W_ih^T[e, g] * x[tau, e] (+bias);
            # full-T moving (N=512 = exactly one PSUM bank), W_ih streamed once.
            for mg in range(NM // 4):
                pans = [wih_pool.tile([P, 4 * P], BF16, tag="wih_t",
                                      name="wih_pan") for _ in range(NK)]
                for kk in range(NK):
                    nc.sync.dma_start(
                        pans[kk][:],
                        wih_d[kk * P:(kk + 1) * P, mg * 4 * P:(mg + 1) * 4 * P])
                for ml in range(4):
                    mp = 4 * mg + ml
                    ps = psA_pool.tile([P, S, BC], F32, tag="psa")
                    for kk in range(NK):
                        nc.tensor.matmul(
                            ps[:], pans[kk][:, ml * P:(ml + 1) * P],
                            xt_sb[:, kk, :],
                            start=(kk == 0), stop=(kk == NK - 1))
                    nc.scalar.activation(
                        xg_sb[:, mp, :, :], ps[:], AF.Identity,
                        bias=bias_sb[:, mp:mp + 1], scale=1.0)

            # ---- recurrence ----
            for t in range(S):
                for hh in range(2):
                    gps = gps_pool.tile([P, 16, BC], F32, tag="gps")
                    # xg injection: I.T @ xgT writes xg_t into the bank exactly
                    # (bf16 identity), with start=True clearing has_written for
                    # the whole bank; the recurrent matmuls below all accumulate
                    # order-independently on top.  Also replaces the DVE add.
                    nc.tensor.matmul(gps[:], ident_sb[:],
                                     xg_sb[:, 16 * hh:16 * hh + 16, t, :],
                                     start=True, stop=(t == 0))
                    # k-loop split in two phases: all m-tiles consume h-tiles 0-3
                    # before any consumes 4-7, so the PE has ~4us of ready work
                    # while the previous step's second-half elementwise finishes.
                    # t=0 skips the recurrence term entirely (h_{-1} == 0).
                    if t > 0:
                        for phase in range(2):
                            for ml in range(16):
                                mp = 16 * hh + ml
                                for kk in range(4 * phase, 4 * phase + 4):
                                    nc.tensor.matmul(
                                        gps[:, ml, :],
                                        whh_sb[:, kk, mp * P:(mp + 1) * P],
                                        h8_sb[:, kk, t - 1, :],
                                        start=False,
                                        stop=(phase == 1 and ml == 15 and kk == NK - 1))
                    gt = gates_pool.tile([P, 16, BC], F32, tag="gt")
                    # block layout per half: [i(0:4), f(4:8), o(8:12), g(12:16)]
                    nc.scalar.activation(gt[:, 0:12, :], gps[:, 0:12, :], AF.Sigmoid)
                    nc.scalar.activation(gt[:, 12:16, :], gps[:, 12:16, :], AF.Tanh)
                    csl = c_sb[:, 4 * hh:4 * hh + 4, :]
                    ig = tmp_pool.tile([P, 4, BC], F32, tag="ig")
                    nc.vector.tensor_mul(ig[:], gt[:, 0:4, :], gt[:, 12:16, :])
                    nc.vector.tensor_mul(csl, gt[:, 4:8, :], csl)
                    nc.vector.tensor_add(csl, csl, ig[:])
                    tc_t = tmp_pool.tile([P, 4, BC], F32, tag="tc_t")
                    nc.scalar.activation(tc_t[:], csl, AF.Tanh)
                    nc.vector.tensor_mul(
                        lstm_sb[:, 4 * hh:4 * hh + 4, t, :], gt[:, 8:12, :], tc_t[:])
                    nc.any.tensor_copy(h8_sb[:, 4 * hh:4 * hh + 4, t, :],
                                       lstm_sb[:, 4 * hh:4 * hh + 4, t, :])

            # ---- combined = lstm_out + features (broadcast over t), bf16 ----
            for kk in range(NK):
                nc.vector.tensor_add(
                    comb_sb[:, kk], lstm_sb[:, kk],
                    feat_sb[:, kk, None, :].to_broadcast([P, S, BC]))

        # ---- GEMM B: out[tau, v] = sum_h combT[h, tau] * out_W^T[h, v] ----
        # 4 vocab tiles per block -> 512KB panel DMAs (4KB contiguous rows);
        # bf16 staged stores halve the output traffic.
        with tc.tile_pool(name="stage", bufs=8) as stage_pool, \
             tc.tile_pool(name="psB", bufs=8, space="PSUM") as psB_pool:
            for bi, blk in enumerate(blocks):
                width = sum(sz for _, sz in blk)
                off0 = blk[0][0]
                if bi in pre_panels:
                    panels = pre_panels[bi]
                else:
                    panels = []
                    for kk in range(NK):
                        pan = outw_pool.tile([P, 4 * 512], BF16, tag="outw_p",
                                             name="outw_p")
                        nc.sync.dma_start(
                            pan[:, :width],
                            outw_d[kk * P:(kk + 1) * P, off0:off0 + width])
                        panels.append(pan)
                for m in range(NMT):
                    pss = [psB_pool.tile([P, 512], F32, tag="psb", name="psb")
                           for _ in blk]
                    for kk in range(NK):
                        for i, (off, sz) in enumerate(blk):
                            po = off - off0
                            nc.tensor.matmul(
                                pss[i][:, :sz],
                                comb_sb[:, kk, 16 * m:16 * (m + 1), :],
                                panels[kk][:, po:po + sz],
                                start=(kk == 0), stop=(kk == NK - 1))
                    for i, (off, sz) in enumerate(blk):
                        st = stage_pool.tile([P, 512], BF16, tag="st")
                        nc.any.tensor_copy(st[:, :sz], pss[i][:, :sz])
                        nc.sync.dma_start(
                            out_d[m * P:(m + 1) * P, off:off + sz], st[:, :sz])


# ------------------------------------------------------------------ host ----


def host_prep(features, captions, embed_table, W_ih, W_hh, b_ih, b_hh,
              out_W, S, V):
    """Shared weights + per-core input shards."""
    bf = ml_dtypes.bfloat16
    b = (np.asarray(b_ih, np.float32) + np.asarray(b_hh, np.float32))[PERM]
    biasT = np.ascontiguousarray(b.reshape(NM, P).T)                # [128, 32]
    # .astype() already yields a C-contiguous copy — no ascontiguousarray pass
    wihT = np.asarray(W_ih, np.float32).T[:, PERM].astype(bf)
    whhT = np.asarray(W_hh, np.float32).T[:, PERM].astype(ml_dtypes.float8_e4m3)
    outwT = np.asarray(out_W, np.float32).T.astype(bf)

    features = np.asarray(features, np.float32)
    cap = np.asarray(captions).astype(np.int64)
    x = np.concatenate(
        [features[:, None, :], np.asarray(embed_table, np.float32)[cap[:, :S - 1]]],
        axis=1)                                                     # (B, S, E)

    shards = []
    B = features.shape[0]
    for c in range(B // BC):
        xc = x[c * BC:(c + 1) * BC]                                 # (8, S, E)
        xT = xc.transpose(2, 1, 0).reshape(H, S * BC).astype(bf)
        fc = features[c * BC:(c + 1) * BC]
        featT = np.ascontiguousarray(fc.T.reshape(NK, P, BC).transpose(1, 0, 2))
        shards.append({"xt": xT, "wih": wihT, "whh": whhT, "outw": outwT,
                       "feat": featT, "biasv": biasT,
                       "ident": np.eye(P, dtype=bf)})
    return shards


def build_program(S, V, reps=1):
    """reps > 1 emits the whole kernel body back-to-back `reps` times in one
    program (used only for timing: amortizes host dispatch overhead)."""
    nc = bacc.Bacc("TRN2", target_bir_lowering=False, debug=False,
                   enable_asserts=False)
    T = S * BC
    io = {
        "xt": nc.dram_tensor("xt", [H, T], BF16, kind="ExternalInput").ap(),
        "wih": nc.dram_tensor("wih", [H, G], BF16, kind="ExternalInput").ap(),
        "whh": nc.dram_tensor("whh", [H, G], F8, kind="ExternalInput").ap(),
        "outw": nc.dram_tensor("outw", [H, V], BF16, kind="ExternalInput").ap(),
        "feat": nc.dram_tensor("feat", [P, NK, BC], F32, kind="ExternalInput").ap(),
        "biasv": nc.dram_tensor("biasv", [P, NM], F32, kind="ExternalInput").ap(),
        "ident": nc.dram_tensor("ident", [P, P], BF16, kind="ExternalInput").ap(),
        "out": nc.dram_tensor("out", [T, V], BF16, kind="ExternalOutput").ap(),
    }
    with tile.TileContext(nc) as tc:
        for _ in range(reps):
            emit_body(tc, io, S, V)
    nc.compile()
    return nc


_CACHE = {}


def _get_program(S, V, reps=1):
    key = (S, V, reps)
    if key not in _CACHE:
        _CACHE[key] = build_program(S, V, reps)
    return _CACHE[key]


def kernel(features, captions, embed_table, W_ih, W_hh, b_ih, b_hh,
           attn_W, attn_b, score_W, score_b, out_W, out_b):
    S = np.asarray(captions).shape[1]
    V = np.asarray(out_W).shape[0]
    B = np.asarray(features).shape[0]
    shards = host_prep(features, captions, embed_table, W_ih, W_hh,
                       b_ih, b_hh, out_W, S, V)
    nc = _get_program(S, V)
    res = run_bass_kernel_spmd(nc, shards, core_ids=list(range(len(shards))))
    out = np.empty((B, S, V), np.float32)
    for c in range(len(shards)):
        oc = np.asarray(res.results[c]["out"]).astype(np.float32)
        out[c * BC:(c + 1) * BC] = oc.reshape(S, BC, V).transpose(1, 0, 2)
    out_b = np.asarray(out_b, np.float32)
    if np.any(out_b):
        out += out_b
    return out

